# revision 2
# baseline (speedup 1.0000x reference)
"""Trainium2 Bass kernel for nn_BMManager: Linear([B,S,1024]->[B,S,512]) + bias,
then per-row segment forward-fill (expand_goals).

Strategy (data-parallel over batch, 8 cores x 4 batch rows each):
  out[r] = x[idx(r)] @ W^T + bias, where idx is the per-row forward-fill index.
  Since the mask is ~p=0.5, only ~half the rows are distinct segment starts.

  Host precomputes (numpy, trivial cost): segment-start list `starts` and
  srcrank[t] = rank of idx(t) in the compact list, both packed into the SWDGE
  int16 index layout.

  Device, per core (R=16384 rows, J_PAD=8704 compact slots):
   1. dma_gather the ~8704 DISTINCT segment-start x rows from HBM (f32,
      4KB each) -- half the HBM read of gathering all 16384 duplicated rows.
   2. cast to bf16 into a resident SBUF x_c [128, 68, 1024].
   3. SBUF-source dma_gather (transpose=True) with idx=srcrank emits the
      duplicated AND pre-transposed x^T tiles [128 d, 8, 512 t] directly --
      no PE transposes at all (saves ~82us of PE time vs the v0 kernel).
   4. 8 accumulating bf16 matmuls per 128-row tile (lhsT=x^T k-slice,
      rhs=W^T k-slice, fp32 PSUM), bias add on DVE, stream rows out.

  PE work is just the GEMM: 1024 matmuls of [128x128]@[128x512].
"""

import numpy as np

import concourse.bacc as bacc
import concourse.bass as bass
import concourse.mybir as mybir
import concourse.tile as tile
from concourse.bass_utils import run_bass_kernel_spmd
from concourse.masks import make_identity

P = 128
N_CORES = 8
B_FULL, S, D_IN, D_GOAL = 32, 4096, 1024, 512
B_PC = B_FULL // N_CORES          # 4 batch rows per core
R = B_PC * S                      # 16384 rows per core
K_TILES = D_IN // P               # 8

J_PAD = 8704                      # compact row slots (mean ~8194, +8 sigma)
NCHUNK = J_PAD // P               # 68 chunks of 128 compact rows
DG_ROWS = 256                     # rows per DRAM gather call
N_DG = J_PAD // DG_ROWS           # 34 calls
SG_ROWS = 512                     # t-rows per SBUF-source gather call
N_SG = R // SG_ROWS               # 32 calls
NQ = 4                            # swdge queues (ucode max)

F32 = mybir.dt.float32
I16 = mybir.dt.int16
BF16 = mybir.dt.bfloat16


def ts(i, n):
    return slice(i * n, (i + 1) * n)


def build_program():
    nc = bacc.Bacc(
        "TRN2",
        target_bir_lowering=False,
        debug=False,
        num_devices=N_CORES,
        num_swdge_queues=NQ,
        use_seq_codegen=True,
    )
    x_d = nc.dram_tensor("x", [R, D_IN], F32, kind="ExternalInput")
    dgidx_d = nc.dram_tensor("dgidx", [P, N_DG * 16], I16, kind="ExternalInput")
    sgidx_d = nc.dram_tensor("sgidx", [P, N_SG * 32], I16, kind="ExternalInput")
    w_d = nc.dram_tensor("w", [D_GOAL, D_IN], F32, kind="ExternalInput")
    bias_d = nc.dram_tensor("bias", [1, D_GOAL], F32, kind="ExternalInput")
    out_d = nc.dram_tensor("out", [R, D_GOAL], F32, kind="ExternalOutput")

    with tile.TileContext(nc) as tc:
        with (
            tc.tile_pool(name="const", bufs=1) as constp,
            tc.tile_pool(name="xs", bufs=2) as xsp,
            tc.tile_pool(name="xc", bufs=1) as xcp,
            tc.tile_pool(name="xt", bufs=3) as xtp,
            tc.tile_pool(name="ys", bufs=3) as ysp,
            tc.tile_pool(name="ptr", bufs=2, space="PSUM") as ptr,
            tc.tile_pool(name="pmm", bufs=5, space="PSUM") as pmm,
        ):
            # ---- constants ----
            ident = constp.tile([P, P], F32)
            make_identity(nc, ident[:])

            dgidx = constp.tile([P, N_DG * 16], I16)
            nc.sync.dma_start(out=dgidx[:], in_=dgidx_d[:])
            sgidx = constp.tile([P, N_SG * 32], I16)
            nc.sync.dma_start(out=sgidx[:], in_=sgidx_d[:])

            bias_ld = constp.tile([1, D_GOAL], F32)
            nc.sync.dma_start(out=bias_ld[:], in_=bias_d[:])
            ones_row = constp.tile([1, P], F32)
            nc.vector.memset(ones_row[:], 1.0)
            psbias = pmm.tile([P, D_GOAL], F32, tag="mm")
            nc.tensor.matmul(
                out=psbias[:], lhsT=ones_row[:], rhs=bias_ld[:], start=True, stop=True
            )
            bias_bc = constp.tile([P, D_GOAL], F32)
            nc.vector.tensor_copy(out=bias_bc[:], in_=psbias[:])

            # ---- W^T: load W [512,1024] (two staged halves), 32 PE transposes ----
            wl0 = xsp.tile([P, 2, D_IN], F32, tag="xs")
            wl1 = xsp.tile([P, 2, D_IN], F32, tag="xs")
            wview = w_d[:].rearrange("(i p) d -> p i d", p=P)
            nc.sync.dma_start(out=wl0[:], in_=wview[:, 0:2, :])
            nc.sync.dma_start(out=wl1[:], in_=wview[:, 2:4, :])
            wt = constp.tile([P, K_TILES * D_GOAL], BF16)
            for k in range(K_TILES):
                psw = ptr.tile([P, D_GOAL], F32, tag="tr")
                for i in range(4):
                    src = wl0 if i < 2 else wl1
                    nc.tensor.transpose(
                        out=psw[:, ts(i, P)],
                        in_=src[:, i % 2, ts(k, P)],
                        identity=ident[:],
                    )
                nc.vector.tensor_copy(out=wt[:, ts(k, D_GOAL)], in_=psw[:])

            # resident compact x in bf16: [128, NCHUNK, 1024]
            xc = xcp.tile([P, NCHUNK, D_IN], BF16)

            def emit_dg(gi):
                """DRAM gather of 256 compact x rows + cast to bf16."""
                xg = xsp.tile([P, DG_ROWS // P, D_IN], F32, tag="xs")
                nc.gpsimd.dma_gather(
                    xg[:],
                    x_d[:],
                    dgidx[:, ts(gi, 16)],
                    num_idxs=DG_ROWS,
                    num_idxs_reg=DG_ROWS,
                    elem_size=D_IN,
                    queue_num=gi % 2,
                )
                dst = xc[:, ts(gi, DG_ROWS // P), :]
                if gi % 2 == 0:
                    nc.vector.tensor_copy(out=dst, in_=xg[:])
                else:
                    nc.scalar.copy(out=dst, in_=xg[:])

            def emit_sg(si):
                """SBUF-source transposing gather of 512 x^T rows + matmuls."""
                ub = min(4 * si + 4, NCHUNK)
                xt = xtp.tile([P, K_TILES, SG_ROWS], BF16)
                nc.gpsimd.dma_gather(
                    xt[:],
                    xc[:, :ub, :],
                    sgidx[:, ts(si, 32)],
                    num_idxs=SG_ROWS,
                    num_idxs_reg=SG_ROWS,
                    elem_size=D_IN,
                    transpose=True,
                    sbuf_tokens_per_rank=P,
                    sbuf_free_dim_per_rank=D_IN * 2,  # bytes per chunk stripe
                    queue_num=2 + si % 2,
                )
                for j in range(SG_ROWS // P):
                    psy = pmm.tile([P, D_GOAL], F32, tag="mm")
                    for k in range(K_TILES):
                        nc.tensor.matmul(
                            out=psy[:],
                            lhsT=xt[:, k, ts(j, P)],
                            rhs=wt[:, ts(k, D_GOAL)],
                            start=(k == 0),
                            stop=(k == K_TILES - 1),
                        )
                    ysb = ysp.tile([P, D_GOAL], F32, tag="ys")
                    nc.vector.tensor_tensor(
                        out=ysb[:], in0=psy[:], in1=bias_bc[:],
                        op=mybir.AluOpType.add,
                    )
                    r0 = si * SG_ROWS + j * P
                    nc.sync.dma_start(out=out_d[r0 : r0 + P, :], in_=ysb[:])

            # interleave: sg(si) needs xc chunks < 4si+4, i.e. dg calls <= 2si+1
            si = 0
            for gi in range(N_DG):
                emit_dg(gi)
                if gi % 2 == 1 and si < N_SG:
                    emit_sg(si)
                    si += 1
            while si < N_SG:
                emit_sg(si)
                si += 1

    nc.compile()
    return nc


_CACHED = {}


def _get_program(**kw):
    key = tuple(sorted(kw.items()))
    if key not in _CACHED:
        _CACHED[key] = build_program(**kw)
    return _CACHED[key]


def _wrap_idx(vals, ncols):
    """Pack a flat index list into the SWDGE [128, ncols] int16 layout:
    element n lives at [n % 16 + 16*rep, n // 16] for all 8 replicas rep."""
    assert vals.size == ncols * 16
    block = vals.reshape(ncols, 16).T.astype(np.int16)  # [16, ncols]
    return np.tile(block, (8, 1))  # [128, ncols]


def make_in_maps(x, critic_mask, W, b):
    x = np.ascontiguousarray(np.asarray(x, dtype=np.float32))
    msk = np.asarray(critic_mask).astype(bool)
    W = np.ascontiguousarray(np.asarray(W, dtype=np.float32))
    b = np.ascontiguousarray(np.asarray(b, dtype=np.float32)).reshape(1, D_GOAL)
    in_maps = []
    for c in range(N_CORES):
        mc = msk[c * B_PC : (c + 1) * B_PC]              # [4, 4096]
        cond = np.ones((B_PC, S), dtype=bool)
        cond[:, 1:] = mc[:, :-1]
        condf = cond.reshape(-1)                          # [16384]
        starts = np.nonzero(condf)[0]                     # [J_c]
        J_c = starts.size
        assert J_c <= J_PAD, f"core {c}: {J_c} segment starts > J_PAD={J_PAD}"
        starts_pad = np.zeros(J_PAD, dtype=np.int64)
        starts_pad[:J_c] = starts
        srcrank = np.cumsum(condf) - 1                    # [16384], <= t
        in_maps.append(
            {
                "x": x[c * B_PC : (c + 1) * B_PC].reshape(R, D_IN),
                "dgidx": _wrap_idx(starts_pad, N_DG * 16),
                "sgidx": _wrap_idx(srcrank, N_SG * 32),
                "w": W,
                "bias": b,
            }
        )
    return in_maps


def kernel(x, critic_mask, W, b, _trace=False, **run_kw):
    nc = _get_program()
    in_maps = make_in_maps(x, critic_mask, W, b)
    res = run_bass_kernel_spmd(
        nc, in_maps, core_ids=list(range(N_CORES)), trace=_trace, **run_kw
    )
    out = np.stack([res.results[c]["out"] for c in range(N_CORES)])
    out = out.reshape(B_FULL, S, D_GOAL)
    if _trace:
        kernel.last_results = res
    return out


if __name__ == "__main__":
    rng = np.random.default_rng(0)
    x = rng.standard_normal((B_FULL, S, D_IN), dtype=np.float32)
    m = rng.integers(0, 2, size=(B_FULL, S)).astype(bool)
    W = rng.standard_normal((D_GOAL, D_IN), dtype=np.float32) / 32.0
    b = rng.standard_normal(D_GOAL).astype(np.float32) * 0.01
    out = kernel(x, m, W, b)
    print(out.shape, out.dtype)


# revision 4
# speedup vs baseline: 1.1075x; 1.1075x over previous
"""Trainium2 Bass kernel for nn_BMManager: Linear([B,S,1024]->[B,S,512]) + bias,
then per-row segment forward-fill (expand_goals).

Strategy (data-parallel over batch, 8 cores x 4 batch rows each):
  out[r] = y[idx(r)], y = x @ W^T + bias, idx = forward-fill index. With a
  p=0.5 mask only ~half the rows are distinct segment starts, so the GEMM
  runs on the COMPACT rows only (J_PAD=8704 slots vs 16384 rows):

  Host (numpy, trivial): segment-start list `starts`, srcrank[t] = rank of
  idx(t) among starts; both packed into the SWDGE int16 index layout.

  Device, per core:
   1. dma_gather the distinct segment-start x rows from HBM (f32 4KB rows,
      half the HBM read of gathering all 16384 duplicated rows).
   2. PE-transpose each 128-row chunk (f32) -> copy to bf16 x^T tiles.
   3. compact GEMM: 8 accumulating bf16 matmuls per chunk -> y_c [j, 512]
      fp32 PSUM; bias add on DVE casts into resident bf16 y_c [128,68,512].
   4. duplication happens on the *output* side: SBUF-source dma_gather
      (transpose=True) with idx=srcrank reads y_c rows (1KB bf16 each) and
      emits y^T tiles [128 g, 4, 512 t]; PE transposes them back to [t, g]
      and scalar/vector copy PSUM->SBUF for the store.

  PE work: 544 GEMM matmuls (half of v0) + ~1k cheap 128-col transposes.
  SBUF-gather traffic is 16.8MB of 1KB rows (vs 33.5MB of 2KB x rows when
  gathering the x side - SBUF-source gathers read a single partition per
  row at only ~10 GB/s/engine, so bytes there are precious).
"""

import numpy as np

import concourse.bacc as bacc
import concourse.bass as bass
import concourse.mybir as mybir
import concourse.tile as tile
from concourse.bass_utils import run_bass_kernel_spmd
from concourse.masks import make_identity

P = 128
N_CORES = 8
B_FULL, S, D_IN, D_GOAL = 32, 4096, 1024, 512
B_PC = B_FULL // N_CORES          # 4 batch rows per core
R = B_PC * S                      # 16384 rows per core
K_TILES = D_IN // P               # 8

J_PAD = 8704                      # compact row slots (mean ~8194, +8 sigma)
NCHUNK = J_PAD // P               # 68 chunks of 128 compact rows
DG_ROWS = 256                     # rows per DRAM gather call
N_DG = J_PAD // DG_ROWS           # 34 calls
SG_ROWS = 512                     # t-rows per SBUF-source gather call
N_SG = R // SG_ROWS               # 32 calls
NQ = 4                            # swdge queues (ucode max)

F32 = mybir.dt.float32
I16 = mybir.dt.int16
BF16 = mybir.dt.bfloat16


def ts(i, n):
    return slice(i * n, (i + 1) * n)


def build_program():
    nc = bacc.Bacc(
        "TRN2",
        target_bir_lowering=False,
        debug=False,
        num_devices=N_CORES,
        num_swdge_queues=NQ,
        use_seq_codegen=True,
    )
    x_d = nc.dram_tensor("x", [R, D_IN], F32, kind="ExternalInput")
    dgidx_d = nc.dram_tensor("dgidx", [P, N_DG * 16], I16, kind="ExternalInput")
    sgidx_d = nc.dram_tensor("sgidx", [P, N_SG * 32], I16, kind="ExternalInput")
    w_d = nc.dram_tensor("w", [D_GOAL, D_IN], F32, kind="ExternalInput")
    bias_d = nc.dram_tensor("bias", [1, D_GOAL], F32, kind="ExternalInput")
    out_d = nc.dram_tensor("out", [R, D_GOAL], F32, kind="ExternalOutput")

    with tile.TileContext(nc) as tc:
        with (
            tc.tile_pool(name="const", bufs=1) as constp,
            tc.tile_pool(name="xs", bufs=3) as xsp,
            tc.tile_pool(name="xt", bufs=3) as xtp,
            tc.tile_pool(name="yc", bufs=1) as ycp,
            tc.tile_pool(name="yt", bufs=3) as ytp,
            tc.tile_pool(name="ys", bufs=4) as ysp,
            tc.tile_pool(name="ptr", bufs=4, space="PSUM") as ptr,
            tc.tile_pool(name="pmm", bufs=2, space="PSUM") as pmm,
            tc.tile_pool(name="pex", bufs=2, space="PSUM") as pex,
        ):
            # ---- constants ----
            ident = constp.tile([P, P], F32)
            make_identity(nc, ident[:])
            ident16 = constp.tile([P, P], BF16)
            make_identity(nc, ident16[:])

            dgidx = constp.tile([P, N_DG * 16], I16)
            nc.sync.dma_start(out=dgidx[:], in_=dgidx_d[:])
            sgidx = constp.tile([P, N_SG * 32], I16)
            nc.sync.dma_start(out=sgidx[:], in_=sgidx_d[:])

            bias_ld = constp.tile([1, D_GOAL], F32)
            nc.sync.dma_start(out=bias_ld[:], in_=bias_d[:])
            ones_row = constp.tile([1, P], F32)
            nc.vector.memset(ones_row[:], 1.0)
            psbias = pmm.tile([P, D_GOAL], F32, tag="mm")
            nc.tensor.matmul(
                out=psbias[:], lhsT=ones_row[:], rhs=bias_ld[:], start=True, stop=True
            )
            bias_bc = constp.tile([P, D_GOAL], F32)
            nc.vector.tensor_copy(out=bias_bc[:], in_=psbias[:])

            # ---- W^T: load W [512,1024] (two staged halves), 32 PE transposes ----
            wl0 = xsp.tile([P, 2, D_IN], F32, tag="xs")
            wl1 = xsp.tile([P, 2, D_IN], F32, tag="xs")
            wview = w_d[:].rearrange("(i p) d -> p i d", p=P)
            nc.sync.dma_start(out=wl0[:], in_=wview[:, 0:2, :])
            nc.sync.dma_start(out=wl1[:], in_=wview[:, 2:4, :])
            wt = constp.tile([P, K_TILES * D_GOAL], BF16)
            for k in range(K_TILES):
                psw = ptr.tile([P, D_GOAL], F32, tag="tr")
                for i in range(4):
                    src = wl0 if i < 2 else wl1
                    nc.tensor.transpose(
                        out=psw[:, ts(i, P)],
                        in_=src[:, i % 2, ts(k, P)],
                        identity=ident[:],
                    )
                nc.vector.tensor_copy(out=wt[:, ts(k, D_GOAL)], in_=psw[:])

            # resident compact y in bf16: [128, NCHUNK, 512]
            yc = ycp.tile([P, NCHUNK, D_GOAL], BF16)

            def emit_dg(gi):
                """DRAM-gather 256 compact x rows; transpose+GEMM both chunks."""
                xg = xsp.tile([P, DG_ROWS // P, D_IN], F32, tag="xs")
                nc.gpsimd.dma_gather(
                    xg[:],
                    x_d[:],
                    dgidx[:, ts(gi, 16)],
                    num_idxs=DG_ROWS,
                    num_idxs_reg=DG_ROWS,
                    elem_size=D_IN,
                    queue_num=gi % 2,
                )
                for h in range(DG_ROWS // P):
                    c = gi * (DG_ROWS // P) + h
                    psA = ptr.tile([P, 4 * P], F32, tag="tr")
                    psB = ptr.tile([P, 4 * P], F32, tag="tr")
                    for k in range(K_TILES):
                        dst = psA if k < 4 else psB
                        nc.tensor.transpose(
                            out=dst[:, ts(k % 4, P)],
                            in_=xg[:, h, ts(k, P)],
                            identity=ident[:],
                        )
                    xt = xtp.tile([P, K_TILES, P], BF16)
                    nc.vector.tensor_copy(
                        out=xt[:, 0:4, :].rearrange("p a b -> p (a b)"), in_=psA[:]
                    )
                    nc.vector.tensor_copy(
                        out=xt[:, 4:8, :].rearrange("p a b -> p (a b)"), in_=psB[:]
                    )
                    psy = pmm.tile([P, D_GOAL], F32, tag="mm")
                    for k in range(K_TILES):
                        nc.tensor.matmul(
                            out=psy[:],
                            lhsT=xt[:, k, :],
                            rhs=wt[:, ts(k, D_GOAL)],
                            start=(k == 0),
                            stop=(k == K_TILES - 1),
                        )
                    nc.vector.tensor_tensor(
                        out=yc[:, c, :], in0=psy[:], in1=bias_bc[:],
                        op=mybir.AluOpType.add,
                    )

            def emit_sg(si):
                """SBUF-source transposing gather of 512 y rows; PE-transpose
                back to [t, g] and store."""
                ub = min(4 * si + 4, NCHUNK)
                yT = ytp.tile([P, D_GOAL // P, SG_ROWS], BF16)
                nc.gpsimd.dma_gather(
                    yT[:],
                    yc[:, :ub, :],
                    sgidx[:, ts(si, 32)],
                    num_idxs=SG_ROWS,
                    num_idxs_reg=SG_ROWS,
                    elem_size=D_GOAL,
                    transpose=True,
                    sbuf_tokens_per_rank=P,
                    sbuf_free_dim_per_rank=D_GOAL * 2,  # bytes per chunk stripe
                    queue_num=2 + si % 2,
                )
                for j in range(SG_ROWS // P):
                    pso = pex.tile([P, D_GOAL], BF16, tag="ex")
                    for gs in range(D_GOAL // P):
                        nc.tensor.transpose(
                            out=pso[:, ts(gs, P)],
                            in_=yT[:, gs, ts(j, P)],
                            identity=ident16[:],
                        )
                    ysb = ysp.tile([P, D_GOAL], F32, tag="ys")
                    if j % 2 == 0:
                        nc.scalar.copy(out=ysb[:], in_=pso[:])
                    else:
                        nc.vector.tensor_copy(out=ysb[:], in_=pso[:])
                    r0 = si * SG_ROWS + j * P
                    nc.sync.dma_start(out=out_d[r0 : r0 + P, :], in_=ysb[:])

            # interleave: sg(si) needs yc chunks < 4si+4, i.e. dg calls <= 2si+1
            si = 0
            for gi in range(N_DG):
                emit_dg(gi)
                if gi % 2 == 1 and si < N_SG:
                    emit_sg(si)
                    si += 1
            while si < N_SG:
                emit_sg(si)
                si += 1

    nc.compile()
    return nc


_CACHED = {}


def _get_program(**kw):
    key = tuple(sorted(kw.items()))
    if key not in _CACHED:
        _CACHED[key] = build_program(**kw)
    return _CACHED[key]


def _wrap_idx(vals, ncols):
    """Pack a flat index list into the SWDGE [128, ncols] int16 layout:
    element n lives at [n % 16 + 16*rep, n // 16] for all 8 replicas rep."""
    assert vals.size == ncols * 16
    block = vals.reshape(ncols, 16).T.astype(np.int16)  # [16, ncols]
    return np.tile(block, (8, 1))  # [128, ncols]


def make_in_maps(x, critic_mask, W, b):
    x = np.ascontiguousarray(np.asarray(x, dtype=np.float32))
    msk = np.asarray(critic_mask).astype(bool)
    W = np.ascontiguousarray(np.asarray(W, dtype=np.float32))
    b = np.ascontiguousarray(np.asarray(b, dtype=np.float32)).reshape(1, D_GOAL)
    in_maps = []
    for c in range(N_CORES):
        mc = msk[c * B_PC : (c + 1) * B_PC]              # [4, 4096]
        cond = np.ones((B_PC, S), dtype=bool)
        cond[:, 1:] = mc[:, :-1]
        condf = cond.reshape(-1)                          # [16384]
        starts = np.nonzero(condf)[0]                     # [J_c]
        J_c = starts.size
        assert J_c <= J_PAD, f"core {c}: {J_c} segment starts > J_PAD={J_PAD}"
        starts_pad = np.zeros(J_PAD, dtype=np.int64)
        starts_pad[:J_c] = starts
        srcrank = np.cumsum(condf) - 1                    # [16384], <= t
        in_maps.append(
            {
                "x": x[c * B_PC : (c + 1) * B_PC].reshape(R, D_IN),
                "dgidx": _wrap_idx(starts_pad, N_DG * 16),
                "sgidx": _wrap_idx(srcrank, N_SG * 32),
                "w": W,
                "bias": b,
            }
        )
    return in_maps


def kernel(x, critic_mask, W, b, _trace=False, **run_kw):
    nc = _get_program()
    in_maps = make_in_maps(x, critic_mask, W, b)
    res = run_bass_kernel_spmd(
        nc, in_maps, core_ids=list(range(N_CORES)), trace=_trace, **run_kw
    )
    out = np.stack([res.results[c]["out"] for c in range(N_CORES)])
    out = out.reshape(B_FULL, S, D_GOAL)
    if _trace:
        kernel.last_results = res
    return out


if __name__ == "__main__":
    rng = np.random.default_rng(0)
    x = rng.standard_normal((B_FULL, S, D_IN), dtype=np.float32)
    m = rng.integers(0, 2, size=(B_FULL, S)).astype(bool)
    W = rng.standard_normal((D_GOAL, D_IN), dtype=np.float32) / 32.0
    b = rng.standard_normal(D_GOAL).astype(np.float32) * 0.01
    out = kernel(x, m, W, b)
    print(out.shape, out.dtype)


# revision 8
# speedup vs baseline: 1.3029x; 1.1764x over previous
"""Trainium2 Bass kernel for nn_BMManager: Linear([B,S,1024]->[B,S,512]) + bias,
then per-row segment forward-fill (expand_goals).

Strategy (data-parallel over batch, 8 cores x 4 batch rows each):
  out[r] = y[idx(r)], y = x @ W^T + bias, idx = forward-fill index. With a
  p=0.5 mask only ~half the rows are distinct segment starts, so the GEMM
  runs on the COMPACT rows only (J_PAD=8704 slots vs 16384 rows):

  Host (numpy, trivial): segment-start list `starts`, srcrank[t] = rank of
  idx(t) among starts; both packed into the SWDGE int16 index layout.

  Device, per core:
   1. dma_gather the distinct segment-start x rows from HBM (f32 4KB rows,
      half the HBM read of gathering all 16384 duplicated rows).
   2. PE-transpose each 128-row chunk (f32) -> copy to bf16 x^T tiles.
   3. compact GEMM: 8 accumulating bf16 matmuls per chunk -> y_c [j, 512]
      fp32 PSUM; bias add on DVE casts into resident bf16 y_c [128,68,512].
   4. duplication happens on the *output* side: SBUF-source dma_gather
      (transpose=True) with idx=srcrank reads y_c rows (1KB bf16 each) and
      emits y^T tiles [128 g, 4, 512 t]; PE transposes them back to [t, g]
      and scalar/vector copy PSUM->SBUF for the store.

  PE work: 544 GEMM matmuls (half of v0) + ~1k cheap 128-col transposes.
  SBUF-gather traffic is 16.8MB of 1KB rows (vs 33.5MB of 2KB x rows when
  gathering the x side - SBUF-source gathers read a single partition per
  row at only ~10 GB/s/engine, so bytes there are precious).
"""

import numpy as np

import concourse.bacc as bacc
import concourse.bass as bass
import concourse.mybir as mybir
import concourse.tile as tile
from concourse.bass_utils import run_bass_kernel_spmd
from concourse.masks import make_identity

P = 128
N_CORES = 8
B_FULL, S, D_IN, D_GOAL = 32, 4096, 1024, 512
B_PC = B_FULL // N_CORES          # 4 batch rows per core
R = B_PC * S                      # 16384 rows per core
K_TILES = D_IN // P               # 8

J_PAD = 8704                      # compact row slots (mean ~8194, +8 sigma)
NCHUNK = J_PAD // P               # 68 chunks of 128 compact rows
DG_ROWS = 256                     # rows per DRAM gather call
N_DG = J_PAD // DG_ROWS           # 34 calls
SG_ROWS = 512                     # t-rows per SBUF-source gather call
N_SG = R // SG_ROWS               # 32 calls
NQ = 4                            # swdge queues (ucode max)

F32 = mybir.dt.float32
I16 = mybir.dt.int16
BF16 = mybir.dt.bfloat16


def ts(i, n):
    return slice(i * n, (i + 1) * n)


def build_program():
    nc = bacc.Bacc(
        "TRN2",
        target_bir_lowering=False,
        debug=False,
        num_devices=N_CORES,
        num_swdge_queues=NQ,
        use_seq_codegen=True,
    )
    x_d = nc.dram_tensor("x", [R, D_IN], F32, kind="ExternalInput")
    dgidx_d = nc.dram_tensor("dgidx", [P, N_DG * 16], I16, kind="ExternalInput")
    sgidx_d = nc.dram_tensor("sgidx", [P, N_SG * 32], I16, kind="ExternalInput")
    w_d = nc.dram_tensor("w", [D_GOAL, D_IN], F32, kind="ExternalInput")
    bias_d = nc.dram_tensor("bias", [1, D_GOAL], F32, kind="ExternalInput")
    out_d = nc.dram_tensor("out", [R, D_GOAL], F32, kind="ExternalOutput")

    with tile.TileContext(nc) as tc:
        with (
            tc.tile_pool(name="const", bufs=1) as constp,
            tc.tile_pool(name="xs", bufs=3) as xsp,
            tc.tile_pool(name="xb", bufs=3) as xbp,
            tc.tile_pool(name="xt", bufs=3) as xtp,
            tc.tile_pool(name="yc", bufs=1) as ycp,
            tc.tile_pool(name="yt", bufs=4) as ytp,
            tc.tile_pool(name="ys", bufs=4) as ysp,
            tc.tile_pool(name="ptr", bufs=3, space="PSUM") as ptr,
            tc.tile_pool(name="pmm", bufs=3, space="PSUM") as pmm,
            tc.tile_pool(name="pex", bufs=2, space="PSUM") as pex,
        ):
            # ---- constants ----
            ident = constp.tile([P, P], F32)
            make_identity(nc, ident[:])
            ident16 = constp.tile([P, P], BF16)
            make_identity(nc, ident16[:])

            dgidx = constp.tile([P, N_DG * 16], I16)
            nc.sync.dma_start(out=dgidx[:], in_=dgidx_d[:])
            sgidx = constp.tile([P, N_SG * 32], I16)
            nc.sync.dma_start(out=sgidx[:], in_=sgidx_d[:])

            bias_ld = constp.tile([1, D_GOAL], F32)
            nc.sync.dma_start(out=bias_ld[:], in_=bias_d[:])
            ones_row = constp.tile([1, P], F32)
            nc.vector.memset(ones_row[:], 1.0)
            psbias = pmm.tile([P, D_GOAL], F32, tag="mm")
            nc.tensor.matmul(
                out=psbias[:], lhsT=ones_row[:], rhs=bias_ld[:], start=True, stop=True
            )
            bias_bc = constp.tile([P, D_GOAL], F32)
            nc.vector.tensor_copy(out=bias_bc[:], in_=psbias[:])

            # ---- W^T: load W [512,1024] (two staged halves), 32 PE transposes ----
            wl0 = xsp.tile([P, 2, D_IN], F32, tag="xs")
            wl1 = xsp.tile([P, 2, D_IN], F32, tag="xs")
            wview = w_d[:].rearrange("(i p) d -> p i d", p=P)
            nc.sync.dma_start(out=wl0[:], in_=wview[:, 0:2, :])
            nc.sync.dma_start(out=wl1[:], in_=wview[:, 2:4, :])
            wt = constp.tile([P, K_TILES * D_GOAL], BF16)
            for k in range(K_TILES):
                psw = ptr.tile([P, D_GOAL], F32, tag="tr")
                for i in range(4):
                    src = wl0 if i < 2 else wl1
                    nc.tensor.transpose(
                        out=psw[:, ts(i, P)],
                        in_=src[:, i % 2, ts(k, P)],
                        identity=ident[:],
                    )
                nc.vector.tensor_copy(out=wt[:, ts(k, D_GOAL)], in_=psw[:])

            # resident compact y in bf16: [128, NCHUNK, 512]
            yc = ycp.tile([P, NCHUNK, D_GOAL], BF16)

            def emit_dg(gi):
                """DRAM-gather 256 compact x rows; cast bf16, transpose+GEMM."""
                xg = xsp.tile([P, DG_ROWS // P, D_IN], F32, tag="xs")
                nc.gpsimd.dma_gather(
                    xg[:],
                    x_d[:],
                    dgidx[:, ts(gi, 16)],
                    num_idxs=DG_ROWS,
                    num_idxs_reg=DG_ROWS,
                    elem_size=D_IN,
                    queue_num=gi % NQ,
                )
                xgb = xbp.tile([P, DG_ROWS // P, D_IN], BF16, tag="xb")
                nc.scalar.copy(out=xgb[:], in_=xg[:])
                for h in range(DG_ROWS // P):
                    c = gi * (DG_ROWS // P) + h
                    psT = ptr.tile([P, K_TILES, P], BF16, tag="tr")
                    for k in range(K_TILES):
                        nc.tensor.transpose(
                            out=psT[:, k, :],
                            in_=xgb[:, h, ts(k, P)],
                            identity=ident16[:],
                        )
                    xt = xtp.tile([P, K_TILES, P], BF16)
                    nc.vector.tensor_copy(
                        out=xt[:].rearrange("p a b -> p (a b)"),
                        in_=psT[:].rearrange("p a b -> p (a b)"),
                    )
                    psy = pmm.tile([P, D_GOAL], F32, tag="mm")
                    for k in range(K_TILES):
                        nc.tensor.matmul(
                            out=psy[:],
                            lhsT=xt[:, k, :],
                            rhs=wt[:, ts(k, D_GOAL)],
                            start=(k == 0),
                            stop=(k == K_TILES - 1),
                        )
                    nc.vector.tensor_tensor(
                        out=yc[:, c, :], in0=psy[:], in1=bias_bc[:],
                        op=mybir.AluOpType.add,
                    )

            def emit_sg(si):
                """SBUF-source transposing gather of 512 y rows; PE-transpose
                back to [t, g] and store."""
                ub = min(4 * si + 4, NCHUNK)
                yT = ytp.tile([P, D_GOAL // P, SG_ROWS], BF16)
                nc.gpsimd.dma_gather(
                    yT[:],
                    yc[:, :ub, :],
                    sgidx[:, ts(si, 32)],
                    num_idxs=SG_ROWS,
                    num_idxs_reg=SG_ROWS,
                    elem_size=D_GOAL,
                    transpose=True,
                    sbuf_tokens_per_rank=P,
                    sbuf_free_dim_per_rank=D_GOAL * 2,  # bytes per chunk stripe
                    queue_num=(si + 2) % NQ,
                )
                for j in range(SG_ROWS // P):
                    pso = pex.tile([P, D_GOAL], BF16, tag="ex")
                    for gs in range(D_GOAL // P):
                        nc.tensor.transpose(
                            out=pso[:, ts(gs, P)],
                            in_=yT[:, gs, ts(j, P)],
                            identity=ident16[:],
                        )
                    ysb = ysp.tile([P, D_GOAL], F32, tag="ys")
                    if j % 2 == 0:
                        nc.scalar.copy(out=ysb[:], in_=pso[:])
                    else:
                        nc.vector.tensor_copy(out=ysb[:], in_=pso[:])
                    r0 = si * SG_ROWS + j * P
                    nc.sync.dma_start(out=out_d[r0 : r0 + P, :], in_=ysb[:])

            # interleave: sg(si) needs yc chunks < 4si+4, i.e. dg calls <= 2si+1.
            # Emit sg(si) after dg call 2si+3 (one extra call of slack) so the
            # semaphore wait guarding the sg descriptor-prep on the serial
            # GpSimd stream is usually already satisfied and does not block
            # later dg preps (head-of-line stall).
            si = 0
            for gi in range(N_DG):
                emit_dg(gi)
                if gi >= 3 and gi % 2 == 1 and si < N_SG:
                    emit_sg(si)
                    si += 1
            while si < N_SG:
                emit_sg(si)
                si += 1

    nc.compile()
    return nc


_CACHED = {}


def _get_program(**kw):
    key = tuple(sorted(kw.items()))
    if key not in _CACHED:
        _CACHED[key] = build_program(**kw)
    return _CACHED[key]


def _wrap_idx(vals, ncols):
    """Pack a flat index list into the SWDGE [128, ncols] int16 layout:
    element n lives at [n % 16 + 16*rep, n // 16] for all 8 replicas rep."""
    assert vals.size == ncols * 16
    block = vals.reshape(ncols, 16).T.astype(np.int16)  # [16, ncols]
    return np.tile(block, (8, 1))  # [128, ncols]


def make_in_maps(x, critic_mask, W, b):
    x = np.ascontiguousarray(np.asarray(x, dtype=np.float32))
    msk = np.asarray(critic_mask).astype(bool)
    W = np.ascontiguousarray(np.asarray(W, dtype=np.float32))
    b = np.ascontiguousarray(np.asarray(b, dtype=np.float32)).reshape(1, D_GOAL)
    in_maps = []
    for c in range(N_CORES):
        mc = msk[c * B_PC : (c + 1) * B_PC]              # [4, 4096]
        cond = np.ones((B_PC, S), dtype=bool)
        cond[:, 1:] = mc[:, :-1]
        condf = cond.reshape(-1)                          # [16384]
        starts = np.nonzero(condf)[0]                     # [J_c]
        J_c = starts.size
        assert J_c <= J_PAD, f"core {c}: {J_c} segment starts > J_PAD={J_PAD}"
        starts_pad = np.zeros(J_PAD, dtype=np.int64)
        starts_pad[:J_c] = starts
        srcrank = np.cumsum(condf) - 1                    # [16384], <= t
        in_maps.append(
            {
                "x": x[c * B_PC : (c + 1) * B_PC].reshape(R, D_IN),
                "dgidx": _wrap_idx(starts_pad, N_DG * 16),
                "sgidx": _wrap_idx(srcrank, N_SG * 32),
                "w": W,
                "bias": b,
            }
        )
    return in_maps


def kernel(x, critic_mask, W, b, _trace=False, **run_kw):
    nc = _get_program()
    in_maps = make_in_maps(x, critic_mask, W, b)
    res = run_bass_kernel_spmd(
        nc, in_maps, core_ids=list(range(N_CORES)), trace=_trace, **run_kw
    )
    out = np.stack([res.results[c]["out"] for c in range(N_CORES)])
    out = out.reshape(B_FULL, S, D_GOAL)
    if _trace:
        kernel.last_results = res
    return out


if __name__ == "__main__":
    rng = np.random.default_rng(0)
    x = rng.standard_normal((B_FULL, S, D_IN), dtype=np.float32)
    m = rng.integers(0, 2, size=(B_FULL, S)).astype(bool)
    W = rng.standard_normal((D_GOAL, D_IN), dtype=np.float32) / 32.0
    b = rng.standard_normal(D_GOAL).astype(np.float32) * 0.01
    out = kernel(x, m, W, b)
    print(out.shape, out.dtype)


# revision 13
# speedup vs baseline: 1.3547x; 1.0398x over previous
"""Trainium2 Bass kernel for nn_BMManager: Linear([B,S,1024]->[B,S,512]) + bias,
then per-row segment forward-fill (expand_goals).

Strategy (data-parallel over batch, 8 cores x 4 batch rows each):
  out[r] = y[idx(r)], y = x @ W^T + bias, idx = forward-fill index. With a
  p=0.5 mask only ~half the rows are distinct segment starts, so the GEMM
  runs on the COMPACT rows only (J_PAD=8704 slots vs 16384 rows):

  Host (numpy, trivial): segment-start list `starts`, srcrank[t] = rank of
  idx(t) among starts; both packed into the SWDGE int16 index layout.

  Device, per core:
   1. dma_gather the distinct segment-start x rows from HBM (f32 4KB rows,
      half the HBM read of gathering all 16384 duplicated rows).
   2. PE-transpose each 128-row chunk (f32) -> copy to bf16 x^T tiles.
   3. compact GEMM: 8 accumulating bf16 matmuls per chunk -> y_c [j, 512]
      fp32 PSUM; bias add on DVE casts into resident bf16 y_c [128,68,512].
   4. duplication happens on the *output* side: SBUF-source dma_gather
      (transpose=True) with idx=srcrank reads y_c rows (1KB bf16 each) and
      emits y^T tiles [128 g, 4, 512 t]; PE transposes them back to [t, g]
      and scalar/vector copy PSUM->SBUF for the store.

  PE work: 544 GEMM matmuls (half of v0) + ~1k cheap 128-col transposes.
  SBUF-gather traffic is 16.8MB of 1KB rows (vs 33.5MB of 2KB x rows when
  gathering the x side - SBUF-source gathers read a single partition per
  row at only ~10 GB/s/engine, so bytes there are precious).
"""

import numpy as np

import concourse.bacc as bacc
import concourse.bass as bass
import concourse.mybir as mybir
import concourse.tile as tile
from concourse.bass_utils import run_bass_kernel_spmd
from concourse.masks import make_identity

P = 128
N_CORES = 8
B_FULL, S, D_IN, D_GOAL = 32, 4096, 1024, 512
B_PC = B_FULL // N_CORES          # 4 batch rows per core
R = B_PC * S                      # 16384 rows per core
K_TILES = D_IN // P               # 8

DG_ROWS = 256                     # rows per DRAM gather call
SG_ROWS = 512                     # t-rows per SBUF-source gather call
N_SG = R // SG_ROWS               # 32 calls
NQ = 4                            # swdge queues (ucode max)

F32 = mybir.dt.float32
I16 = mybir.dt.int16
BF16 = mybir.dt.bfloat16


def ts(i, n):
    return slice(i * n, (i + 1) * n)


def build_program(j_pad, ub_list):
    """j_pad: compact slots (multiple of 256). ub_list[si]: yc chunk upper
    bound needed by SBUF-gather window si (data-derived; = max srcrank in the
    window // 128 + 1). Baking the true bound keeps late sg descriptor preps
    off the final GEMM chunks' critical path."""
    NCHUNK = j_pad // P
    N_DG = j_pad // DG_ROWS
    nc = bacc.Bacc(
        "TRN2",
        target_bir_lowering=False,
        debug=False,
        num_devices=N_CORES,
        num_swdge_queues=NQ,
        use_seq_codegen=True,
    )
    x_d = nc.dram_tensor("x", [R, D_IN], F32, kind="ExternalInput")
    dgidx_d = nc.dram_tensor("dgidx", [P, N_DG * 16], I16, kind="ExternalInput")
    sgidx_d = nc.dram_tensor("sgidx", [P, N_SG * 32], I16, kind="ExternalInput")
    w_d = nc.dram_tensor("w", [D_GOAL, D_IN], F32, kind="ExternalInput")
    bias_d = nc.dram_tensor("bias", [1, D_GOAL], F32, kind="ExternalInput")
    out_d = nc.dram_tensor("out", [R, D_GOAL], F32, kind="ExternalOutput")

    with tile.TileContext(nc) as tc:
        with (
            tc.tile_pool(name="const", bufs=1) as constp,
            tc.tile_pool(name="xs", bufs=3) as xsp,
            tc.tile_pool(name="xb", bufs=3) as xbp,
            tc.tile_pool(name="xt", bufs=3) as xtp,
            tc.tile_pool(name="yc", bufs=1) as ycp,
            tc.tile_pool(name="yt", bufs=4) as ytp,
            tc.tile_pool(name="ys", bufs=4) as ysp,
            tc.tile_pool(name="ptr", bufs=3, space="PSUM") as ptr,
            tc.tile_pool(name="pmm", bufs=3, space="PSUM") as pmm,
            tc.tile_pool(name="pex", bufs=2, space="PSUM") as pex,
        ):
            # ---- constants ----
            ident = constp.tile([P, P], F32)
            make_identity(nc, ident[:])
            ident16 = constp.tile([P, P], BF16)
            make_identity(nc, ident16[:])

            dgidx = constp.tile([P, N_DG * 16], I16)
            nc.sync.dma_start(out=dgidx[:], in_=dgidx_d[:])
            sgidx = constp.tile([P, N_SG * 32], I16)
            nc.sync.dma_start(out=sgidx[:], in_=sgidx_d[:])

            bias_ld = constp.tile([1, D_GOAL], F32)
            nc.sync.dma_start(out=bias_ld[:], in_=bias_d[:])
            ones_row = constp.tile([1, P], F32)
            nc.vector.memset(ones_row[:], 1.0)
            psbias = pmm.tile([P, D_GOAL], F32, tag="mm")
            nc.tensor.matmul(
                out=psbias[:], lhsT=ones_row[:], rhs=bias_ld[:], start=True, stop=True
            )
            bias_bc = constp.tile([P, D_GOAL], F32)
            nc.vector.tensor_copy(out=bias_bc[:], in_=psbias[:])

            # ---- W^T: load W [512,1024] (two staged halves), 32 PE transposes ----
            wl0 = xsp.tile([P, 2, D_IN], F32, tag="xs")
            wl1 = xsp.tile([P, 2, D_IN], F32, tag="xs")
            wview = w_d[:].rearrange("(i p) d -> p i d", p=P)
            nc.sync.dma_start(out=wl0[:], in_=wview[:, 0:2, :])
            nc.sync.dma_start(out=wl1[:], in_=wview[:, 2:4, :])
            wt = constp.tile([P, K_TILES * D_GOAL], BF16)
            for k in range(K_TILES):
                psw = ptr.tile([P, D_GOAL], F32, tag="tr")
                for i in range(4):
                    src = wl0 if i < 2 else wl1
                    nc.tensor.transpose(
                        out=psw[:, ts(i, P)],
                        in_=src[:, i % 2, ts(k, P)],
                        identity=ident[:],
                    )
                nc.vector.tensor_copy(out=wt[:, ts(k, D_GOAL)], in_=psw[:])

            # resident compact y in bf16: [128, NCHUNK, 512]
            yc = ycp.tile([P, NCHUNK, D_GOAL], BF16)

            def emit_dg(gi):
                """DRAM-gather 256 compact x rows; cast bf16, transpose+GEMM."""
                xg = xsp.tile([P, DG_ROWS // P, D_IN], F32, tag="xs")
                nc.gpsimd.dma_gather(
                    xg[:],
                    x_d[:],
                    dgidx[:, ts(gi, 16)],
                    num_idxs=DG_ROWS,
                    num_idxs_reg=DG_ROWS,
                    elem_size=D_IN,
                    queue_num=gi % NQ,
                )
                xgb = xbp.tile([P, DG_ROWS // P, D_IN], BF16, tag="xb")
                nc.scalar.copy(out=xgb[:], in_=xg[:])
                for h in range(DG_ROWS // P):
                    c = gi * (DG_ROWS // P) + h
                    psT = ptr.tile([P, K_TILES, P], BF16, tag="tr")
                    for k in range(K_TILES):
                        nc.tensor.transpose(
                            out=psT[:, k, :],
                            in_=xgb[:, h, ts(k, P)],
                            identity=ident16[:],
                        )
                    xt = xtp.tile([P, K_TILES, P], BF16)
                    nc.vector.tensor_copy(
                        out=xt[:].rearrange("p a b -> p (a b)"),
                        in_=psT[:].rearrange("p a b -> p (a b)"),
                    )
                    psy = pmm.tile([P, D_GOAL], F32, tag="mm")
                    for k in range(K_TILES):
                        nc.tensor.matmul(
                            out=psy[:],
                            lhsT=xt[:, k, :],
                            rhs=wt[:, ts(k, D_GOAL)],
                            start=(k == 0),
                            stop=(k == K_TILES - 1),
                        )
                    nc.vector.tensor_tensor(
                        out=yc[:, c, :], in0=psy[:], in1=bias_bc[:],
                        op=mybir.AluOpType.add,
                    )

            def emit_sg(si):
                """SBUF-source transposing gather of 512 y rows; PE-transpose
                back to [t, g] and store."""
                ub = min(ub_list[si], NCHUNK)
                yT = ytp.tile([P, D_GOAL // P, SG_ROWS], BF16)
                nc.gpsimd.dma_gather(
                    yT[:],
                    yc[:, :ub, :],
                    sgidx[:, ts(si, 32)],
                    num_idxs=SG_ROWS,
                    num_idxs_reg=SG_ROWS,
                    elem_size=D_GOAL,
                    transpose=True,
                    sbuf_tokens_per_rank=P,
                    sbuf_free_dim_per_rank=D_GOAL * 2,  # bytes per chunk stripe
                    queue_num=(si + 2) % NQ,
                )
                for j in range(SG_ROWS // P):
                    pso = pex.tile([P, D_GOAL], BF16, tag="ex")
                    for gs in range(D_GOAL // P):
                        nc.tensor.transpose(
                            out=pso[:, ts(gs, P)],
                            in_=yT[:, gs, ts(j, P)],
                            identity=ident16[:],
                        )
                    ysb = ysp.tile([P, D_GOAL], F32, tag="ys")
                    if j % 2 == 0:
                        nc.scalar.copy(out=ysb[:], in_=pso[:])
                    else:
                        nc.vector.tensor_copy(out=ysb[:], in_=pso[:])
                    r0 = si * SG_ROWS + j * P
                    nc.sync.dma_start(out=out_d[r0 : r0 + P, :], in_=ysb[:])

            # interleave: sg(si) needs yc chunks < ub_list[si], i.e. dg calls
            # <= ceil(ub/2)-1. Emit sg(si) with one extra dg call of slack so
            # the semaphore wait guarding the sg descriptor-prep on the serial
            # GpSimd stream is usually already satisfied and does not block
            # later dg preps (head-of-line stall).
            si = 0
            for gi in range(N_DG):
                emit_dg(gi)
                while si < N_SG and gi >= (min(ub_list[si], NCHUNK) + 1) // 2 + 1:
                    emit_sg(si)
                    si += 1
            while si < N_SG:
                emit_sg(si)
                si += 1

    nc.compile()
    return nc


_CACHED = {}


def _get_program(**kw):
    key = tuple(sorted(kw.items()))
    if key not in _CACHED:
        _CACHED[key] = build_program(**kw)
    return _CACHED[key]


def _wrap_idx(vals, ncols):
    """Pack a flat index list into the SWDGE [128, ncols] int16 layout:
    element n lives at [n % 16 + 16*rep, n // 16] for all 8 replicas rep."""
    assert vals.size == ncols * 16
    block = vals.reshape(ncols, 16).T.astype(np.int16)  # [16, ncols]
    return np.tile(block, (8, 1))  # [128, ncols]


def _core_indices(critic_mask):
    """Per-core (starts, srcrank) from the mask."""
    msk = np.asarray(critic_mask).astype(bool)
    per_core = []
    for c in range(N_CORES):
        mc = msk[c * B_PC : (c + 1) * B_PC]              # [4, 4096]
        cond = np.ones((B_PC, S), dtype=bool)
        cond[:, 1:] = mc[:, :-1]
        condf = cond.reshape(-1)                          # [16384]
        starts = np.nonzero(condf)[0]                     # [J_c]
        srcrank = np.cumsum(condf) - 1                    # [16384], <= t
        per_core.append((starts, srcrank))
    return per_core


def _compute_meta(per_core):
    """(j_pad, ub_list) from the actual index data."""
    j_max = max(s.size for s, _ in per_core)
    j_pad = -(-j_max // DG_ROWS) * DG_ROWS               # round up to 256
    ub_list = []
    for si in range(N_SG):
        t_hi = (si + 1) * SG_ROWS - 1
        m = max(int(r[t_hi]) for _, r in per_core)       # srcrank nondecreasing
        ub_list.append(m // P + 1)
    return j_pad, tuple(ub_list)


def make_in_maps(x, critic_mask, W, b, per_core, j_pad):
    x = np.ascontiguousarray(np.asarray(x, dtype=np.float32))
    W = np.ascontiguousarray(np.asarray(W, dtype=np.float32))
    b = np.ascontiguousarray(np.asarray(b, dtype=np.float32)).reshape(1, D_GOAL)
    n_dg = j_pad // DG_ROWS
    in_maps = []
    for c in range(N_CORES):
        starts, srcrank = per_core[c]
        starts_pad = np.zeros(j_pad, dtype=np.int64)
        starts_pad[: starts.size] = starts
        in_maps.append(
            {
                "x": x[c * B_PC : (c + 1) * B_PC].reshape(R, D_IN),
                "dgidx": _wrap_idx(starts_pad, n_dg * 16),
                "sgidx": _wrap_idx(srcrank, N_SG * 32),
                "w": W,
                "bias": b,
            }
        )
    return in_maps


def kernel(x, critic_mask, W, b, _trace=False, **run_kw):
    per_core = _core_indices(critic_mask)
    j_pad, ub_list = _compute_meta(per_core)
    nc = _get_program(j_pad=j_pad, ub_list=ub_list)
    in_maps = make_in_maps(x, critic_mask, W, b, per_core, j_pad)
    res = run_bass_kernel_spmd(
        nc, in_maps, core_ids=list(range(N_CORES)), trace=_trace, **run_kw
    )
    out = np.stack([res.results[c]["out"] for c in range(N_CORES)])
    out = out.reshape(B_FULL, S, D_GOAL)
    if _trace:
        kernel.last_results = res
    return out


if __name__ == "__main__":
    rng = np.random.default_rng(0)
    x = rng.standard_normal((B_FULL, S, D_IN), dtype=np.float32)
    m = rng.integers(0, 2, size=(B_FULL, S)).astype(bool)
    W = rng.standard_normal((D_GOAL, D_IN), dtype=np.float32) / 32.0
    b = rng.standard_normal(D_GOAL).astype(np.float32) * 0.01
    out = kernel(x, m, W, b)
    print(out.shape, out.dtype)
